# revision 11
# baseline (speedup 1.0000x reference)
"""CBOW negative-sampling loss kernel for Trainium2 (8 NeuronCores).

Problem: nn_CBOWModel_18356690223611
    pos_u  [16384, 10] int  -- context word ids into u_weight
    pos_w  [16384]     int  -- target word ids into w_weight
    neg_w  [16384, 5]  int  -- negative sample ids into w_weight
    u_weight [100000, 128] f32
    w_weight [100000, 128] f32
    out = sum_b softplus(-dot(su_b, wpos_b)) + softplus(dot(su_b, wneg_sum_b))
      where su_b = sum_c u_weight[pos_u[b,c]], wneg_sum_b = sum_k w_weight[neg_w[b,k]]
    (equivalent to -(sum logsigmoid(pos) + sum logsigmoid(-neg)))

Sharding: data-parallel over batch, 2048 samples per core; embedding tables
replicated (concatenated into one [200000, 128] DRAM tensor) per core.

The gather is 256 SWDGE indirect DMAs per core. This is descgen-bound: the
indirect1d ISA consumes exactly one index per dest partition (its 1D tensor
descriptors cannot express per-row descriptors across partitions), and this
image's firmware runs all SWDGE descgen on one Q7 pair (~1.12us per
instruction, serial; multi-queue round-robin measured no speedup, HWDGE RTL
explicitly excludes indirection, and the extended-ISA dma_gather ucode is
not present on this image). So the kernel keeps the descgen stream dense and
hides everything else under it:
  - all 256 gathers land in one 128KB/partition SBUF tile (no buffer reuse
    -> no WAR stalls),
  - the index tile is preloaded on the sync queue's HWDGE (chunk-0 columns
    in a first small DMA so descgen starts as early as possible),
  - DVE tree-sums/dots, ACT softplus and the PE cross-partition sum trail
    the gather stream per chunk; the last chunks are narrowed (4,4,4,2,2
    sample-columns) so almost no compute remains after the final gather.
"""

import numpy as np

VOCAB = 100000
DIM = 128
B = 16384
CTX = 10
NEG = 5
WK = NEG + 1  # pos + neg lookups into w_weight per sample
NIDX = CTX + WK  # 16 gathered rows per sample

N_CORES = 8
BPC = B // N_CORES  # 2048 samples per core
P = 128
TILES = BPC // P  # 16 sample columns of 128 samples
CHUNK_WIDTHS = (4, 4, 4, 3, 1)  # sample columns per pipeline chunk
NS = 2 * TILES  # score columns: (pos, neg) per sample column

_CACHE = {}


def _build_nc():
    import concourse.bacc as bacc
    import concourse.bass as bass
    import concourse.mybir as mybir
    import concourse.tile as tile

    f32 = mybir.dt.float32
    i32 = mybir.dt.int32
    ADD = mybir.AluOpType.add
    MUL = mybir.AluOpType.mult

    nc = bacc.Bacc("TRN2", target_bir_lowering=False, debug=False,
                   enable_asserts=False)

    idx_d = nc.dram_tensor("idx", [P, NIDX * TILES], i32,
                           kind="ExternalInput")
    uw_w = nc.dram_tensor("uw_weight", [2 * VOCAB, DIM], f32,
                          kind="ExternalInput")
    out_d = nc.dram_tensor("out", [1, 1], f32, kind="ExternalOutput")

    with tile.TileContext(nc) as tc:
        with (
            tc.tile_pool(name="idx", bufs=1) as idxp,
            tc.tile_pool(name="g", bufs=1) as gpool,
            tc.tile_pool(name="work", bufs=2) as work,
            tc.tile_pool(name="accum", bufs=1) as accp,
            tc.tile_pool(name="psum", bufs=1, space="PSUM") as psp,
        ):
            idx_t = idxp.tile([P, NIDX * TILES], i32)
            # idx load on the gpsimd (Pool) queue itself: measured ~1.6us
            # faster to first gather than the sync HWDGE path; chunk-0
            # columns in a first small DMA so descgen starts sooner
            c0 = NIDX * CHUNK_WIDTHS[0]
            nc.gpsimd.dma_start(out=idx_t[:, 0:c0], in_=idx_d.ap()[:, 0:c0])
            nc.gpsimd.dma_start(out=idx_t[:, c0:NIDX * TILES],
                                in_=idx_d.ap()[:, c0:NIDX * TILES])

            # one gather tile for the whole batch: 256 rows per partition
            g_t = gpool.tile([P, NIDX * TILES * DIM], f32)

            # scores[p, :]: per chunk, W pos-score cols then W neg-score cols
            scores = accp.tile([P, NS], f32)

            # partial softplus: row_out[p] = sum_i softplus(cols[p, i]),
            # overflow-safe: softplus(x) = relu(x) + log1p(exp(-|x|))
            def softplus_rowsum(cols, n, tag):
                relu = accp.tile([P, n], f32, tag=f"relu{tag}")
                nc.vector.tensor_scalar_max(relu[:], cols, 0.0)
                nabs = accp.tile([P, n], f32, tag=f"nabs{tag}")
                nc.vector.scalar_tensor_tensor(  # -|x| = x - 2*relu
                    out=nabs[:], in0=relu[:], scalar=-2.0, in1=cols,
                    op0=MUL, op1=ADD)
                ex = accp.tile([P, n], f32, tag=f"ex{tag}")
                nc.scalar.activation(ex[:], nabs[:],
                                     mybir.ActivationFunctionType.Exp)
                ln = accp.tile([P, n], f32, tag=f"ln{tag}")
                nc.scalar.activation(ln[:], ex[:],
                                     mybir.ActivationFunctionType.Ln, bias=1.0)
                sp = accp.tile([P, n], f32, tag=f"sp{tag}")
                nc.vector.tensor_tensor(out=sp[:], in0=relu[:], in1=ln[:], op=ADD)
                row = accp.tile([P, 1], f32, tag=f"row{tag}")
                nc.vector.tensor_reduce(out=row[:], in_=sp[:],
                                        axis=mybir.AxisListType.X, op=ADD)
                return row

            row0 = None
            base = 0  # gather-block / idx column offset
            soff = 0  # score column offset
            for ci, W in enumerate(CHUNK_WIDTHS):
                blks = NIDX * W
                # one gather per (c, t) block; host orders idx columns
                # chunk-major, then c-major, t-minor; u rows then pos_w/neg_w
                for j in range(blks):
                    col = base + j
                    nc.gpsimd.indirect_dma_start(
                        out=g_t[:, col * DIM:(col + 1) * DIM],
                        out_offset=None,
                        in_=uw_w.ap(),
                        in_offset=bass.IndirectOffsetOnAxis(
                            ap=idx_t[:, col:col + 1], axis=0),
                    )
                u4 = g_t[:, base * DIM:(base + CTX * W) * DIM].rearrange(
                    "p (c t d) -> p c t d", c=CTX, t=W)
                w4 = g_t[:, (base + CTX * W) * DIM:(base + blks) * DIM].rearrange(
                    "p (c t d) -> p c t d", c=WK, t=W)

                # context sum over c=10: tree 10 -> 5 -> (4->2->1) + leftover
                s1 = work.tile([P, 5 * W * DIM], f32, tag="s1")
                s1v = s1[:].rearrange("p (c t d) -> p c t d", c=5, t=W)
                nc.vector.tensor_tensor(out=s1v[:, :, :, :], in0=u4[:, 0:5], in1=u4[:, 5:10], op=ADD)
                s2 = work.tile([P, 2 * W * DIM], f32, tag="s2")
                s2v = s2[:].rearrange("p (c t d) -> p c t d", c=2, t=W)
                nc.vector.tensor_tensor(out=s2v[:, :, :, :], in0=s1v[:, 0:2], in1=s1v[:, 2:4], op=ADD)
                s3 = work.tile([P, W * DIM], f32, tag="s3")
                s3v = s3[:].rearrange("p (o t d) -> p o t d", o=1, t=W)
                nc.vector.tensor_tensor(out=s3v[:, :, :, :], in0=s2v[:, 0:1], in1=s2v[:, 1:2], op=ADD)
                su = work.tile([P, W * DIM], f32, tag="su")
                suv = su[:].rearrange("p (o t d) -> p o t d", o=1, t=W)
                nc.vector.tensor_tensor(out=suv[:, :, :, :], in0=s3v[:, :, :, :], in1=s1v[:, 4:5], op=ADD)

                # negative-sample sum over c=1..5: 4 -> 2 -> 1, + leftover
                n1 = work.tile([P, 2 * W * DIM], f32, tag="n1")
                n1v = n1[:].rearrange("p (c t d) -> p c t d", c=2, t=W)
                nc.vector.tensor_tensor(out=n1v[:, :, :, :], in0=w4[:, 1:3], in1=w4[:, 3:5], op=ADD)
                n2 = work.tile([P, W * DIM], f32, tag="n2")
                n2v = n2[:].rearrange("p (o t d) -> p o t d", o=1, t=W)
                nc.vector.tensor_tensor(out=n2v[:, :, :, :], in0=n1v[:, 0:1], in1=n1v[:, 1:2], op=ADD)
                wneg = work.tile([P, W * DIM], f32, tag="wneg")
                wnv = wneg[:].rearrange("p (o t d) -> p o t d", o=1, t=W)
                nc.vector.tensor_tensor(out=wnv[:, :, :, :], in0=n2v[:, :, :, :], in1=w4[:, 5:6], op=ADD)

                # per-sample dot products
                prod = work.tile([P, 2 * W * DIM], f32, tag="prod")
                pv = prod[:].rearrange("p (k t d) -> p k t d", k=2, t=W)
                nc.vector.tensor_tensor(out=pv[:, 0:1], in0=suv[:, :, :, :], in1=w4[:, 0:1], op=MUL)
                nc.vector.tensor_tensor(out=pv[:, 1:2], in0=suv[:, :, :, :], in1=wnv[:, :, :, :], op=MUL)
                sv = scores[:, soff:soff + 2 * W].rearrange(
                    "p (k t) -> p k t", k=2)
                nc.vector.tensor_reduce(
                    out=sv[:, 0:1, :], in_=pv[:, 0:1],
                    axis=mybir.AxisListType.X, op=ADD, negate=True)
                nc.vector.tensor_reduce(
                    out=sv[:, 1:2, :], in_=pv[:, 1:2],
                    axis=mybir.AxisListType.X, op=ADD)
                base += blks
                soff += 2 * W
                if ci == len(CHUNK_WIDTHS) - 2:
                    # all but the last chunk's scores are final: fold them
                    # through softplus now, hidden under the last gathers
                    row0 = softplus_rowsum(scores[:, 0:soff], soff, "0")

            # tail: only the last chunk's score columns remain
            n1c = NS - 2 * CHUNK_WIDTHS[-1]
            row1 = softplus_rowsum(scores[:, n1c:NS], NS - n1c, "1")
            row = accp.tile([P, 1], f32, tag="rowsum")
            nc.vector.tensor_tensor(out=row[:], in0=row0[:], in1=row1[:], op=ADD)

            # cross-partition sum: [1,1] = row.T @ ones
            ones = accp.tile([P, 1], f32)
            nc.vector.memset(ones[:], 1.0)
            ps = psp.tile([1, 1], f32)
            nc.tensor.matmul(ps[:], lhsT=row[:], rhs=ones[:], start=True, stop=True)
            res_sb = accp.tile([1, 1], f32)
            nc.vector.tensor_copy(out=res_sb[:], in_=ps[:])
            nc.sync.dma_start(out=out_d.ap(), in_=res_sb[:])

    # Exp and Ln both live in the natural_log_exp_and_others table set, but
    # the greedy table chooser picks exp_and_others for Exp and natural_log
    # for Ln, putting a ~2.7us table swap in the kernel's serial tail. Empty
    # those two sets (positions preserved -- act_func_set_id is positional)
    # during compile so both funcs resolve to the combined table.
    orig_tables = bacc.get_activation_tables

    def _tables_combined(arch):
        t = dict(orig_tables(arch))
        if "natural_log_exp_and_others" in t:
            for k in ("exp_and_others", "natural_log"):
                if k in t:
                    t[k] = frozenset()
        return t

    bacc.get_activation_tables = _tables_combined
    try:
        nc.compile()
    finally:
        bacc.get_activation_tables = orig_tables
    return nc


def _get_nc():
    if "nc" not in _CACHE:
        _CACHE["nc"] = _build_nc()
    return _CACHE["nc"]


def _make_in_maps(pos_u, pos_w, neg_w, u_weight, w_weight):
    pos_u = np.asarray(pos_u)
    pos_w = np.asarray(pos_w)
    neg_w = np.asarray(neg_w)
    uw = np.ascontiguousarray(
        np.concatenate([np.asarray(u_weight, dtype=np.float32),
                        np.asarray(w_weight, dtype=np.float32)], axis=0))

    in_maps = []
    for c in range(N_CORES):
        sl = slice(c * BPC, (c + 1) * BPC)
        # per-sample 16 indices: u c=0..9 then w k=0..5 (+VOCAB offset into
        # the concatenated table)
        all_ind = np.concatenate(
            [np.asarray(pos_u[sl], dtype=np.int32),
             np.asarray(pos_w[sl], dtype=np.int32)[:, None] + VOCAB,
             np.asarray(neg_w[sl], dtype=np.int32) + VOCAB], axis=1)  # [2048, 16]
        # device layout: columns chunk-major, within a chunk of width W the
        # column for lookup j of sample s = (t0 + t)*128 + p is j*W + t
        A = all_ind.reshape(TILES, P, NIDX)  # [t_global, p, j]
        cols = []
        t0 = 0
        for W in CHUNK_WIDTHS:
            blk = A[t0:t0 + W]                      # [W, p, j]
            cols.append(blk.transpose(1, 2, 0).reshape(P, NIDX * W))
            t0 += W
        idx = np.concatenate(cols, axis=1)  # [P, NIDX*TILES]
        in_maps.append({
            "idx": np.ascontiguousarray(idx),
            "uw_weight": uw,
        })
    return in_maps


def kernel(pos_u, pos_w, neg_w, u_weight, w_weight):
    from concourse.bass_utils import run_bass_kernel_spmd

    nc = _get_nc()
    in_maps = _make_in_maps(pos_u, pos_w, neg_w, u_weight, w_weight)
    res = run_bass_kernel_spmd(nc, in_maps, core_ids=list(range(N_CORES)))
    total = sum(float(r["out"][0, 0]) for r in res.results)
    return np.asarray(total, dtype=np.float32)


# revision 15
# speedup vs baseline: 1.0088x; 1.0088x over previous
"""CBOW negative-sampling loss kernel for Trainium2 (8 NeuronCores).

Problem: nn_CBOWModel_18356690223611
    pos_u  [16384, 10] int  -- context word ids into u_weight
    pos_w  [16384]     int  -- target word ids into w_weight
    neg_w  [16384, 5]  int  -- negative sample ids into w_weight
    u_weight [100000, 128] f32
    w_weight [100000, 128] f32
    out = sum_b softplus(-dot(su_b, wpos_b)) + softplus(dot(su_b, wneg_sum_b))
      where su_b = sum_c u_weight[pos_u[b,c]], wneg_sum_b = sum_k w_weight[neg_w[b,k]]
    (equivalent to -(sum logsigmoid(pos) + sum logsigmoid(-neg)))

Sharding: data-parallel over batch, 2048 samples per core; embedding tables
replicated (concatenated into one [200000, 128] DRAM tensor) per core.

The gather is 256 SWDGE indirect DMAs per core. This is descgen-bound: the
indirect1d ISA consumes exactly one index per dest partition (its 1D tensor
descriptors cannot express per-row descriptors across partitions), and this
image's firmware runs all SWDGE descgen on one Q7 pair (~1.12us per
instruction, serial; multi-queue round-robin measured no speedup, HWDGE RTL
explicitly excludes indirection, and the extended-ISA dma_gather ucode is
not present on this image). So the kernel keeps the descgen stream dense and
hides everything else under it:
  - all 256 gathers land in one 128KB/partition SBUF tile (no buffer reuse
    -> no WAR stalls),
  - the index tile is preloaded on the sync queue's HWDGE (chunk-0 columns
    in a first small DMA so descgen starts as early as possible),
  - DVE tree-sums/dots, ACT softplus and the PE cross-partition sum trail
    the gather stream per chunk; the last chunks are narrowed (4,4,4,2,2
    sample-columns) so almost no compute remains after the final gather.
"""

import numpy as np

VOCAB = 100000
DIM = 128
B = 16384
CTX = 10
NEG = 5
WK = NEG + 1  # pos + neg lookups into w_weight per sample
NIDX = CTX + WK  # 16 gathered rows per sample

N_CORES = 8
BPC = B // N_CORES  # 2048 samples per core
P = 128
TILES = BPC // P  # 16 sample columns of 128 samples
CHUNK_WIDTHS = (4, 4, 4, 2, 2)  # sample columns per pipeline chunk
NS = 2 * TILES  # score columns: (pos, neg) per sample column

_CACHE = {}


def _build_nc():
    import concourse.bacc as bacc
    import concourse.bass as bass
    import concourse.mybir as mybir
    import concourse.tile as tile

    f32 = mybir.dt.float32
    i32 = mybir.dt.int32
    ADD = mybir.AluOpType.add
    MUL = mybir.AluOpType.mult

    nc = bacc.Bacc("TRN2", target_bir_lowering=False, debug=False,
                   enable_asserts=False)

    idx_d = nc.dram_tensor("idx", [P, NIDX * TILES], i32,
                           kind="ExternalInput")
    uw_w = nc.dram_tensor("uw_weight", [2 * VOCAB, DIM], f32,
                          kind="ExternalInput")
    out_d = nc.dram_tensor("out", [1, 1], f32, kind="ExternalOutput")

    with tile.TileContext(nc) as tc:
        with (
            tc.tile_pool(name="idx", bufs=1) as idxp,
            tc.tile_pool(name="g", bufs=1) as gpool,
            tc.tile_pool(name="work", bufs=2) as work,
            tc.tile_pool(name="accum", bufs=1) as accp,
            tc.tile_pool(name="psum", bufs=1, space="PSUM") as psp,
        ):
            idx_t = idxp.tile([P, NIDX * TILES], i32)
            # idx load on the sync queue's HWDGE; chunk-0 columns in a first
            # small DMA so gather descgen can start as soon as they land
            c0 = NIDX * CHUNK_WIDTHS[0]
            nc.sync.dma_start(out=idx_t[:, 0:c0], in_=idx_d.ap()[:, 0:c0])
            nc.sync.dma_start(out=idx_t[:, c0:NIDX * TILES],
                              in_=idx_d.ap()[:, c0:NIDX * TILES])

            # one gather tile for the whole batch: 256 rows per partition
            g_t = gpool.tile([P, NIDX * TILES * DIM], f32)

            # scores[p, :]: per chunk, W pos-score cols then W neg-score cols
            scores = accp.tile([P, NS], f32)

            base = 0  # gather-block / idx column offset
            soff = 0  # score column offset
            for ci, W in enumerate(CHUNK_WIDTHS):
                blks = NIDX * W
                # one gather per (c, t) block; host orders idx columns
                # chunk-major, then c-major, t-minor; u rows then pos_w/neg_w
                for j in range(blks):
                    col = base + j
                    nc.gpsimd.indirect_dma_start(
                        out=g_t[:, col * DIM:(col + 1) * DIM],
                        out_offset=None,
                        in_=uw_w.ap(),
                        in_offset=bass.IndirectOffsetOnAxis(
                            ap=idx_t[:, col:col + 1], axis=0),
                    )
                u4 = g_t[:, base * DIM:(base + CTX * W) * DIM].rearrange(
                    "p (c t d) -> p c t d", c=CTX, t=W)
                w4 = g_t[:, (base + CTX * W) * DIM:(base + blks) * DIM].rearrange(
                    "p (c t d) -> p c t d", c=WK, t=W)

                # context sum over c=10: tree 10 -> 5 -> (4->2->1) + leftover
                s1 = work.tile([P, 5 * W * DIM], f32, tag="s1")
                s1v = s1[:].rearrange("p (c t d) -> p c t d", c=5, t=W)
                nc.vector.tensor_tensor(out=s1v[:, :, :, :], in0=u4[:, 0:5], in1=u4[:, 5:10], op=ADD)
                s2 = work.tile([P, 2 * W * DIM], f32, tag="s2")
                s2v = s2[:].rearrange("p (c t d) -> p c t d", c=2, t=W)
                nc.vector.tensor_tensor(out=s2v[:, :, :, :], in0=s1v[:, 0:2], in1=s1v[:, 2:4], op=ADD)
                s3 = work.tile([P, W * DIM], f32, tag="s3")
                s3v = s3[:].rearrange("p (o t d) -> p o t d", o=1, t=W)
                nc.vector.tensor_tensor(out=s3v[:, :, :, :], in0=s2v[:, 0:1], in1=s2v[:, 1:2], op=ADD)
                su = work.tile([P, W * DIM], f32, tag="su")
                suv = su[:].rearrange("p (o t d) -> p o t d", o=1, t=W)
                nc.vector.tensor_tensor(out=suv[:, :, :, :], in0=s3v[:, :, :, :], in1=s1v[:, 4:5], op=ADD)

                # negative-sample sum over c=1..5: 4 -> 2 -> 1, + leftover
                n1 = work.tile([P, 2 * W * DIM], f32, tag="n1")
                n1v = n1[:].rearrange("p (c t d) -> p c t d", c=2, t=W)
                nc.vector.tensor_tensor(out=n1v[:, :, :, :], in0=w4[:, 1:3], in1=w4[:, 3:5], op=ADD)
                n2 = work.tile([P, W * DIM], f32, tag="n2")
                n2v = n2[:].rearrange("p (o t d) -> p o t d", o=1, t=W)
                nc.vector.tensor_tensor(out=n2v[:, :, :, :], in0=n1v[:, 0:1], in1=n1v[:, 1:2], op=ADD)
                wneg = work.tile([P, W * DIM], f32, tag="wneg")
                wnv = wneg[:].rearrange("p (o t d) -> p o t d", o=1, t=W)
                nc.vector.tensor_tensor(out=wnv[:, :, :, :], in0=n2v[:, :, :, :], in1=w4[:, 5:6], op=ADD)

                # per-sample dot products
                prod = work.tile([P, 2 * W * DIM], f32, tag="prod")
                pv = prod[:].rearrange("p (k t d) -> p k t d", k=2, t=W)
                nc.vector.tensor_tensor(out=pv[:, 0:1], in0=suv[:, :, :, :], in1=w4[:, 0:1], op=MUL)
                nc.vector.tensor_tensor(out=pv[:, 1:2], in0=suv[:, :, :, :], in1=wnv[:, :, :, :], op=MUL)
                sv = scores[:, soff:soff + 2 * W].rearrange(
                    "p (k t) -> p k t", k=2)
                nc.vector.tensor_reduce(
                    out=sv[:, 0:1, :], in_=pv[:, 0:1],
                    axis=mybir.AxisListType.X, op=ADD, negate=True)
                nc.vector.tensor_reduce(
                    out=sv[:, 1:2, :], in_=pv[:, 1:2],
                    axis=mybir.AxisListType.X, op=ADD)
                base += blks
                soff += 2 * W

            # tail: res = sum_{p,i} softplus(scores[p,i]), overflow-safe:
            # softplus(x) = relu(x) + log1p(exp(-|x|))
            relu = accp.tile([P, NS], f32)
            nc.vector.tensor_scalar_max(relu[:], scores[:], 0.0)
            nabs = accp.tile([P, NS], f32)
            nc.vector.scalar_tensor_tensor(  # -|x| = scores - 2*relu
                out=nabs[:], in0=relu[:], scalar=-2.0, in1=scores[:],
                op0=MUL, op1=ADD)
            ex = accp.tile([P, NS], f32)
            nc.scalar.activation(ex[:], nabs[:], mybir.ActivationFunctionType.Exp)
            ln = accp.tile([P, NS], f32)
            nc.scalar.activation(ln[:], ex[:], mybir.ActivationFunctionType.Ln,
                                 bias=1.0)
            sp = accp.tile([P, NS], f32)
            nc.vector.tensor_tensor(out=sp[:], in0=relu[:], in1=ln[:], op=ADD)
            row = accp.tile([P, 1], f32)
            nc.vector.tensor_reduce(out=row[:], in_=sp[:],
                                    axis=mybir.AxisListType.X, op=ADD)

            # cross-partition sum: [1,1] = row.T @ ones
            ones = accp.tile([P, 1], f32)
            nc.vector.memset(ones[:], 1.0)
            ps = psp.tile([1, 1], f32)
            nc.tensor.matmul(ps[:], lhsT=row[:], rhs=ones[:], start=True, stop=True)
            res_sb = accp.tile([1, 1], f32)
            nc.vector.tensor_copy(out=res_sb[:], in_=ps[:])
            nc.sync.dma_start(out=out_d.ap(), in_=res_sb[:])

    # Exp and Ln both live in the natural_log_exp_and_others table set, but
    # the greedy table chooser picks exp_and_others for Exp and natural_log
    # for Ln, putting a ~2.7us table swap in the kernel's serial tail. Empty
    # those two sets (positions preserved -- act_func_set_id is positional)
    # during compile so both funcs resolve to the combined table.
    orig_tables = bacc.get_activation_tables

    def _tables_combined(arch):
        t = dict(orig_tables(arch))
        if "natural_log_exp_and_others" in t:
            for k in ("exp_and_others", "natural_log"):
                if k in t:
                    t[k] = frozenset()
        return t

    bacc.get_activation_tables = _tables_combined
    try:
        nc.compile()
    finally:
        bacc.get_activation_tables = orig_tables
    return nc


def _get_nc():
    if "nc" not in _CACHE:
        _CACHE["nc"] = _build_nc()
    return _CACHE["nc"]


def _make_in_maps(pos_u, pos_w, neg_w, u_weight, w_weight):
    pos_u = np.asarray(pos_u)
    pos_w = np.asarray(pos_w)
    neg_w = np.asarray(neg_w)
    uw = np.ascontiguousarray(
        np.concatenate([np.asarray(u_weight, dtype=np.float32),
                        np.asarray(w_weight, dtype=np.float32)], axis=0))

    in_maps = []
    for c in range(N_CORES):
        sl = slice(c * BPC, (c + 1) * BPC)
        # per-sample 16 indices: u c=0..9 then w k=0..5 (+VOCAB offset into
        # the concatenated table)
        all_ind = np.concatenate(
            [np.asarray(pos_u[sl], dtype=np.int32),
             np.asarray(pos_w[sl], dtype=np.int32)[:, None] + VOCAB,
             np.asarray(neg_w[sl], dtype=np.int32) + VOCAB], axis=1)  # [2048, 16]
        # device layout: columns chunk-major, within a chunk of width W the
        # column for lookup j of sample s = (t0 + t)*128 + p is j*W + t
        A = all_ind.reshape(TILES, P, NIDX)  # [t_global, p, j]
        cols = []
        t0 = 0
        for W in CHUNK_WIDTHS:
            blk = A[t0:t0 + W]                      # [W, p, j]
            cols.append(blk.transpose(1, 2, 0).reshape(P, NIDX * W))
            t0 += W
        idx = np.concatenate(cols, axis=1)  # [P, NIDX*TILES]
        in_maps.append({
            "idx": np.ascontiguousarray(idx),
            "uw_weight": uw,
        })
    return in_maps


def kernel(pos_u, pos_w, neg_w, u_weight, w_weight):
    from concourse.bass_utils import run_bass_kernel_spmd

    nc = _get_nc()
    in_maps = _make_in_maps(pos_u, pos_w, neg_w, u_weight, w_weight)
    res = run_bass_kernel_spmd(nc, in_maps, core_ids=list(range(N_CORES)))
    total = sum(float(r["out"][0, 0]) for r in res.results)
    return np.asarray(total, dtype=np.float32)
